# revision 7
# baseline (speedup 1.0000x reference)
"""Trainium2 8-core Bass kernel for a dual cross-attention transformer block.

v2 sharding: QKV projections are computed token-locally (each core projects
its own 512 tokens for ALL 16 heads), then one AllToAll per side moves Q/K/V
into head-parallel layout (2 heads per core, all 4096 tokens) for attention.
A second (small) AllToAll per side moves the attention outputs back to
token-parallel layout, after which Wo, the residual, and the whole FFN are
core-local.  This replaces the baseline's AllGather(activations) +
ReduceScatter(Wo partials) — 67MB of wire traffic — with ~14.5MB of AllToAll
traffic.  Compute dtype bf16 (FFN weights fp8 DoubleRow), fp32 PSUM.
"""

import math

import numpy as np
import ml_dtypes

import concourse.bass as bass
import concourse.tile as tile
from concourse import mybir, bacc
from concourse.bass_utils import run_bass_kernel_spmd

B, S, D, H = 2, 2048, 2048, 16
HD = D // H            # 128
HID = 5632
EPS = 1e-5
NC = 8                 # cores
HPC = H // NC          # 2 heads per core (attention phase)
T = B * S              # 4096 tokens
TPC = T // NC          # 512 tokens per core
KCH = D // 128         # 16 contraction chunks over D
JCH = HID // 128       # 44 chunks over HID
BF = ml_dtypes.bfloat16
E4 = ml_dtypes.float8_e4m3
F32 = mybir.dt.float32
BF16 = mybir.dt.bfloat16
FP8 = mybir.dt.float8e4
import os as _os
USE_FP8 = _os.environ.get("BASS_FP8", "1") == "1"
USE_FP8_ATT = _os.environ.get("BASS_FP8_ATT", "1") == "1"
FSC = 16.0
EXPC = 2.0             # exp(s - EXPC) keeps fp8 probs in range

_CACHE = {}


def _rope_perm():
    # [evens, odds]: puts x1 in partitions 0:64, x2 in 64:128 of Q^T/K^T
    return np.concatenate([np.arange(0, 128, 2), np.arange(1, 128, 2)])


def build_nc(sim_local=False, reps=1, fp8=None, fp8a=None):
    if fp8 is None:
        fp8 = USE_FP8
    if fp8a is None:
        fp8a = USE_FP8_ATT
    adt = FP8 if fp8a else BF16       # attention-path dtype (proj/transport)
    nc = bacc.Bacc("TRN2", target_bir_lowering=False, debug=False,
                   num_devices=1 if sim_local else NC)

    AL = mybir.AluOpType
    AF = mybir.ActivationFunctionType

    # ---------------- external parameters ----------------
    xT = {s: nc.declare_dram_parameter(f"{s}T", [D, TPC], F32, isOutput=False)
          for s in ("x", "y")}
    # per-core rope table slices, stacked [c;c] and [-s;s]; q pre-scaled
    tabs = {n: nc.declare_dram_parameter(n, [128, TPC], F32, isOutput=False)
            for n in ("cs_q", "sn_q", "cs_k", "sn_k")}
    wq, wk, wv, wo = {}, {}, {}, {}
    bq, bqs, bk, bks, bv, bo = {}, {}, {}, {}, {}, {}
    w1, w3, w2, b1, b3, b2, fnw = {}, {}, {}, {}, {}, {}, {}
    for s in ("x", "y"):
        wq[s] = nc.declare_dram_parameter(f"wq_{s}", [H, 128, KCH, 128], BF16, isOutput=False)
        wk[s] = nc.declare_dram_parameter(f"wk_{s}", [H, 128, KCH, 128], BF16, isOutput=False)
        wv[s] = nc.declare_dram_parameter(f"wv_{s}", [4, 128, KCH, 512], BF16, isOutput=False)
        wo[s] = nc.declare_dram_parameter(f"wo_{s}", [KCH, 128, KCH, 128], BF16, isOutput=False)
        bq[s] = nc.declare_dram_parameter(f"bq_{s}", [H, 128], F32, isOutput=False)
        bqs[s] = nc.declare_dram_parameter(f"bqs_{s}", [H, 128], F32, isOutput=False)
        bk[s] = nc.declare_dram_parameter(f"bk_{s}", [H, 128], F32, isOutput=False)
        bks[s] = nc.declare_dram_parameter(f"bks_{s}", [H, 128], F32, isOutput=False)
        bv[s] = nc.declare_dram_parameter(f"bv_{s}", [D], F32, isOutput=False)
        bo[s] = nc.declare_dram_parameter(f"bo_{s}", [KCH, 128], F32, isOutput=False)
        wdt = FP8 if fp8 else BF16
        w1[s] = nc.declare_dram_parameter(f"w1_{s}", [JCH, 128, KCH, 128], wdt, isOutput=False)
        w3[s] = nc.declare_dram_parameter(f"w3_{s}", [JCH, 128, KCH, 128], wdt, isOutput=False)
        w2[s] = nc.declare_dram_parameter(f"w2_{s}", [KCH, 128, JCH, 128], wdt, isOutput=False)
        b1[s] = nc.declare_dram_parameter(f"b1_{s}", [JCH, 128], F32, isOutput=False)
        b3[s] = nc.declare_dram_parameter(f"b3_{s}", [JCH, 128], F32, isOutput=False)
        b2[s] = nc.declare_dram_parameter(f"b2_{s}", [KCH, 128], F32, isOutput=False)
        fnw[s] = nc.declare_dram_parameter(f"fnw_{s}", [KCH, 128], F32, isOutput=False)
    anw = nc.declare_dram_parameter("anw", [KCH, 128], F32, isOutput=False)
    out_ext = nc.declare_dram_parameter("out", [2, D, TPC], F32, isOutput=True)

    # ---------------- internal DRAM (AllToAll bounce buffers) ----------------
    # qkv block for dest core d: rows 0:256 Q^T heads {2d,2d+1}; 256:512 K^T;
    # 512:768 V packed as [head-parity, tok-chunk, 128 tok, 128 hd].
    qkv_in = {s: nc.dram_tensor(f"qkv_in_{s}", [NC, 768, TPC], BF16)
              for s in ("x", "y")}
    qkv_out = {s: nc.dram_tensor(f"qkv_out_{s}", [NC, 768, TPC], BF16)
               for s in ("x", "y")}
    o_in = {s: nc.dram_tensor(f"o_in_{s}", [NC, HPC * 128, TPC], BF16)
            for s in ("x", "y")}
    o_out = {s: nc.dram_tensor(f"o_out_{s}", [NC, HPC * 128, TPC], BF16)
             for s in ("x", "y")}

    rg = [list(range(NC))]

    def a2a(in_t, out_t):
        if sim_local:
            for g in range(NC):
                nc.sync.dma_start(out=out_t[g], in_=in_t[g])
        else:
            nc.gpsimd.collective_compute(
                "AllToAll", AL.bypass, replica_groups=rg,
                ins=[in_t[:]], outs=[out_t[:]])

    from contextlib import ExitStack
    with tile.TileContext(nc) as tc:
        with ExitStack() as es:
            const = es.enter_context(tc.tile_pool(name="const", bufs=1))
            ones_bf = const.tile([128, 1], BF16)
            nc.vector.memset(ones_bf, 1.0)
            ones_f = const.tile([128, 1], F32)
            nc.vector.memset(ones_f, 1.0)
            ones_row = const.tile([1, 128], F32)
            nc.vector.memset(ones_row, 1.0)
            eps_sb = const.tile([128, 1], F32)
            nc.vector.memset(eps_sb, EPS)
            anw_sb = const.tile([128, KCH], F32)
            nc.sync.dma_start(out=anw_sb, in_=anw.rearrange("k p -> p k"))
            tb = {}
            for n in ("cs_q", "sn_q", "cs_k", "sn_k"):
                tb[n] = const.tile([128, TPC], F32, name=n, tag=n)
                nc.sync.dma_start(out=tb[n], in_=tabs[n][:])
            fnw_sb, bo_sb, b2_sb, b1_sb, b3_sb = {}, {}, {}, {}, {}
            bq_sb, bqs_sb, bk_sb, bks_sb, bv_sb = {}, {}, {}, {}, {}
            for s in ("x", "y"):
                fnw_sb[s] = const.tile([128, KCH], F32, name=f"fnw{s}", tag=f"fnw{s}")
                nc.sync.dma_start(out=fnw_sb[s], in_=fnw[s].rearrange("k p -> p k"))
                bo_sb[s] = const.tile([128, KCH], F32, name=f"bo{s}", tag=f"bo{s}")
                nc.sync.dma_start(out=bo_sb[s], in_=bo[s].rearrange("k p -> p k"))
                b2_sb[s] = const.tile([128, KCH], F32, name=f"b2{s}", tag=f"b2{s}")
                nc.sync.dma_start(out=b2_sb[s], in_=b2[s].rearrange("k p -> p k"))
                b1_sb[s] = const.tile([128, JCH], F32, name=f"b1{s}", tag=f"b1{s}")
                nc.sync.dma_start(out=b1_sb[s], in_=b1[s].rearrange("k p -> p k"))
                b3_sb[s] = const.tile([128, JCH], F32, name=f"b3{s}", tag=f"b3{s}")
                nc.sync.dma_start(out=b3_sb[s], in_=b3[s].rearrange("k p -> p k"))
                bq_sb[s] = const.tile([128, H], F32, name=f"bq{s}", tag=f"bq{s}")
                nc.sync.dma_start(out=bq_sb[s], in_=bq[s].rearrange("h p -> p h"))
                bqs_sb[s] = const.tile([128, H], F32, name=f"bqs{s}", tag=f"bqs{s}")
                nc.sync.dma_start(out=bqs_sb[s], in_=bqs[s].rearrange("h p -> p h"))
                bk_sb[s] = const.tile([128, H], F32, name=f"bk{s}", tag=f"bk{s}")
                nc.sync.dma_start(out=bk_sb[s], in_=bk[s].rearrange("h p -> p h"))
                bks_sb[s] = const.tile([128, H], F32, name=f"bks{s}", tag=f"bks{s}")
                nc.sync.dma_start(out=bks_sb[s], in_=bks[s].rearrange("h p -> p h"))
                bv_sb[s] = const.tile([128, D], F32, name=f"bv{s}", tag=f"bv{s}")
                nc.sync.dma_start(out=bv_sb[s],
                                  in_=bv[s][None, :].to_broadcast([128, D]))

            for _rep in range(reps):
                # ---------- phase 1: rms-norm own 512 tokens, both sides ----
                nrm_es = ExitStack()
                ntp = nrm_es.enter_context(tc.tile_pool(name="ntp", bufs=1))
                nt = {}
                with tc.tile_pool(name="nrm", bufs=2) as nrm, \
                     tc.tile_pool(name="nrm_ps", bufs=2, space="PSUM") as nrm_ps:
                    for s in ("x", "y"):
                        xt_sb = []
                        ms_ps = nrm_ps.tile([1, TPC], F32, name="ms", tag="ms")
                        for kc in range(KCH):
                            t = nrm.tile([128, TPC], F32, name="xt", tag="xt", bufs=18)
                            nc.sync.dma_start(out=t, in_=xT[s][kc * 128:(kc + 1) * 128, :])
                            xt_sb.append(t)
                            sq = nrm.tile([128, TPC], F32, name="sq", tag="sq")
                            nc.scalar.activation(out=sq, in_=t, func=AF.Square)
                            nc.tensor.matmul(ms_ps[:, 0:TPC], ones_f, sq,
                                             start=(kc == 0), stop=(kc == KCH - 1))
                        sd = nrm.tile([1, TPC], F32, name="sd", tag="sd")
                        nc.scalar.activation(out=sd, in_=ms_ps, func=AF.Sqrt,
                                             bias=eps_sb[0:1, :], scale=1.0 / D)
                        rec = nrm.tile([1, TPC], F32, name="rec", tag="rec")
                        nc.vector.reciprocal(out=rec, in_=sd)
                        rb_ps = nrm_ps.tile([128, TPC], F32, name="rb", tag="rb")
                        nc.tensor.matmul(rb_ps, ones_row, rec, start=True, stop=True)
                        rb = nrm.tile([128, TPC], F32, name="rbs", tag="rbs")
                        nc.scalar.copy(out=rb, in_=rb_ps)
                        for kc in range(KCH):
                            nti = ntp.tile([128, TPC], BF16, name=f"nt{s}{kc}",
                                           tag=f"nt{s}{kc}")
                            nc.vector.scalar_tensor_tensor(
                                out=nti, in0=xt_sb[kc], scalar=anw_sb[:, kc:kc + 1],
                                in1=rb, op0=AL.mult, op1=AL.mult)
                            nt[(s, kc)] = nti

                # ---------- phase 2: QKV for all heads on own tokens + A2A ----
                with tc.tile_pool(name="qw", bufs=1) as qw, \
                     tc.tile_pool(name="rp", bufs=3) as rp, \
                     tc.tile_pool(name="qkv_ps", bufs=2, space="PSUM") as qkv_ps:
                    for s in ("x", "y"):
                        qside = "y" if s == "x" else "x"   # queries from hidden
                        for hh in range(H):
                            for proj, wt, bsb, bssb, cs_t, sn_t, rowbase in (
                                ("q", wq[s], bq_sb[s], bqs_sb[s],
                                 tb["cs_q"], tb["sn_q"], 0),
                                ("k", wk[s], bk_sb[s], bks_sb[s],
                                 tb["cs_k"], tb["sn_k"], 256),
                            ):
                                src = qside if proj == "q" else s
                                w_sb = qw.tile([128, KCH, 128], BF16,
                                               name=f"w{proj}", tag=f"w{proj}", bufs=3)
                                nc.sync.dma_start(out=w_sb, in_=wt[hh])
                                ps = qkv_ps.tile([128, TPC], F32, name="qk", tag="qk")
                                for kc in range(KCH):
                                    nc.tensor.matmul(ps, w_sb[:, kc, :],
                                                     nt[(src, kc)],
                                                     start=(kc == 0),
                                                     stop=(kc == KCH - 1))
                                qs = rp.tile([128, TPC], F32, name="qs", tag="qs")
                                nc.scalar.copy(out=qs, in_=ps)
                                qsw = rp.tile([128, TPC], F32, name="qsw", tag="qsw")
                                nc.sync.dma_start(out=qsw[0:64, :], in_=qs[64:128, :])
                                nc.sync.dma_start(out=qsw[64:128, :], in_=qs[0:64, :])
                                t1 = rp.tile([128, TPC], F32, name="t1", tag="t1")
                                nc.vector.scalar_tensor_tensor(
                                    out=t1, in0=qs, scalar=bsb[:, hh:hh + 1],
                                    in1=cs_t, op0=AL.add, op1=AL.mult)
                                t2 = rp.tile([128, TPC], F32, name="t2", tag="t2")
                                nc.vector.scalar_tensor_tensor(
                                    out=t2, in0=qsw, scalar=bssb[:, hh:hh + 1],
                                    in1=sn_t, op0=AL.add, op1=AL.mult)
                                dst = rp.tile([128, TPC], BF16, name="dst", tag="dst")
                                nc.vector.tensor_add(dst, t1, t2)
                                rb0 = rowbase + (hh % 2) * 128
                                nc.sync.dma_start(
                                    out=qkv_in[s][hh // 2, rb0:rb0 + 128, :],
                                    in_=dst)
                        # V: natural [token, hd] layout, two dest cores per pass
                        for dp in range(4):
                            wv_sb = qw.tile([128, KCH, 512], BF16,
                                            name="wv", tag="wv", bufs=2)
                            nc.sync.dma_start(out=wv_sb, in_=wv[s][dp])
                            for tk in range(4):
                                vps = qkv_ps.tile([128, 512], F32, name="v", tag="v")
                                for kc in range(KCH):
                                    nc.tensor.matmul(
                                        vps, nt[(s, kc)][:, tk * 128:(tk + 1) * 128],
                                        wv_sb[:, kc, :],
                                        start=(kc == 0), stop=(kc == KCH - 1))
                                vsb = rp.tile([128, 512], BF16, name="vsb", tag="vsb")
                                nc.vector.tensor_add(
                                    vsb, vps, bv_sb[s][:, dp * 512:(dp + 1) * 512])
                                for e2 in range(4):
                                    d = 2 * dp + e2 // 2
                                    e = e2 % 2
                                    r0 = 512 + e * 128 + tk * 32
                                    nc.sync.dma_start(
                                        out=qkv_in[s][d, r0:r0 + 32, :].rearrange(
                                            "r (u h) -> (r u) h", u=4),
                                        in_=vsb[:, e2 * 128:(e2 + 1) * 128])
                        a2a(qkv_in[s], qkv_out[s])
                nrm_es.close()

                # ---------- phase 3: attention per side (2 heads/core) ----
                with tc.tile_pool(name="ktqt", bufs=1) as ktqt:
                    for s in ("x", "y"):
                        qt_sb, kt_sb = {}, {}
                        for h in range(HPC):
                            qt_sb[h] = ktqt.tile([128, T], BF16, name=f"qt{h}",
                                                 tag=f"qt{h}", bufs=2)
                            kt_sb[h] = ktqt.tile([128, T], BF16, name=f"kt{h}",
                                                 tag=f"kt{h}", bufs=2)
                            for g in range(NC):
                                nc.sync.dma_start(
                                    out=qt_sb[h][:, g * TPC:(g + 1) * TPC],
                                    in_=qkv_out[s][g, h * 128:(h + 1) * 128, :])
                                nc.sync.dma_start(
                                    out=kt_sb[h][:, g * TPC:(g + 1) * TPC],
                                    in_=qkv_out[s][g, 256 + h * 128:
                                                   256 + (h + 1) * 128, :])
                        with tc.tile_pool(name="att", bufs=2) as att, \
                             tc.tile_pool(name="att_pt", bufs=2) as att_pt, \
                             tc.tile_pool(name="att_ps", bufs=2, space="PSUM") as att_ps, \
                             tc.tile_pool(name="att_ps1", bufs=1, space="PSUM") as att_ps1:
                            for b in range(B):
                                for h in range(HPC):
                                    vsl = att.tile([128, KCH, 128], BF16,
                                                   name="vsl", tag="vsl")
                                    for g4 in range(4):
                                        g = b * 4 + g4
                                        for tk in range(4):
                                            r0 = 512 + h * 128 + tk * 32
                                            nc.sync.dma_start(
                                                out=vsl[:, g4 * 4 + tk, :],
                                                in_=qkv_out[s][g, r0:r0 + 32, :]
                                                .rearrange("r (u hh) -> (r u) hh",
                                                           u=4))
                                    for blk in range(S // 1024):
                                        tq0 = b * S + blk * 1024
                                        pt = []
                                        for tkc in range(16):
                                            sps = att_ps.tile([128, 1024], F32,
                                                              name="s", tag="s")
                                            for i in (0, 1):
                                                nc.tensor.matmul(
                                                    sps[:, i * 512:(i + 1) * 512],
                                                    kt_sb[h][:, b * S + tkc * 128:
                                                             b * S + (tkc + 1) * 128],
                                                    qt_sb[h][:, tq0 + i * 512:
                                                             tq0 + (i + 1) * 512],
                                                    start=True, stop=True)
                                            p = att_pt.tile([128, 1024], BF16,
                                                            name="pt", tag="pt",
                                                            bufs=20)
                                            nc.scalar.activation(out=p, in_=sps,
                                                                 func=AF.Exp)
                                            pt.append(p)
                                        # denominator: DVE pair-tree then
                                        # ones-matmul partition sum
                                        lvl = pt
                                        li = 0
                                        while len(lvl) > 1:
                                            nxt = []
                                            for i in range(0, len(lvl), 2):
                                                dsum = att_pt.tile(
                                                    [128, 1024], BF16,
                                                    name="dsum", tag=f"ds{li}", bufs=3)
                                                nc.vector.tensor_add(dsum, lvl[i],
                                                                     lvl[i + 1])
                                                nxt.append(dsum)
                                            lvl = nxt
                                            li += 1
                                        ops = att_ps1.tile([128, 1024], F32,
                                                           name="ops", tag="ops")
                                        den = att_ps1.tile([1, 1024], F32,
                                                           name="den", tag="aux")
                                        for i in (0, 1):
                                            nc.tensor.matmul(
                                                den[:, i * 512:(i + 1) * 512], ones_bf,
                                                lvl[0][:, i * 512:(i + 1) * 512],
                                                start=True, stop=True)
                                        for tkc in range(16):
                                            for i in (0, 1):
                                                nc.tensor.matmul(
                                                    ops[:, i * 512:(i + 1) * 512],
                                                    vsl[:, tkc, :],
                                                    pt[tkc][:, i * 512:(i + 1) * 512],
                                                    start=(tkc == 0), stop=(tkc == 15))
                                        rec = att.tile([1, 1024], F32, name="rec",
                                                       tag="rec")
                                        nc.vector.reciprocal(out=rec, in_=den)
                                        rb = att_ps1.tile([128, 1024], F32,
                                                          name="rb", tag="aux")
                                        for i in (0, 1):
                                            nc.tensor.matmul(
                                                rb[:, i * 512:(i + 1) * 512],
                                                ones_row,
                                                rec[:, i * 512:(i + 1) * 512],
                                                start=True, stop=True)
                                        rbs = att.tile([128, 1024], F32, name="rbs",
                                                       tag="rbs")
                                        nc.scalar.copy(out=rbs, in_=rb)
                                        ot = att.tile([128, 1024], BF16, name="ot",
                                                      tag="ot", bufs=3)
                                        nc.vector.tensor_mul(ot, ops, rbs)
                                        g0 = tq0 // TPC
                                        for gg in (0, 1):
                                            nc.sync.dma_start(
                                                out=o_in[s][g0 + gg,
                                                            h * 128:(h + 1) * 128, :],
                                                in_=ot[:, gg * TPC:(gg + 1) * TPC])
                        a2a(o_in[s], o_out[s])

                # ---------- phase 4: Wo + FFN + residual + norm per side ----
                for s in ("x", "y"):
                    side_es = ExitStack()
                    ffn_h = side_es.enter_context(
                        tc.tile_pool(name="ffn_h", bufs=1))
                    hT = []
                    h8 = [ffn_h.tile([128, 2, TPC], FP8, name=f"h8_{i}",
                                     tag=f"h8_{i}") for i in range(KCH // 2)] \
                        if fp8 else None
                    with tc.tile_pool(name="osb", bufs=1) as osb, \
                         tc.tile_pool(name="wo_w", bufs=3) as wo_w, \
                         tc.tile_pool(name="wo_ps", bufs=2, space="PSUM") as wo_ps:
                        # assemble o^T [D, TPC] (head-major) from o-A2A
                        o_sb = []
                        for kc in range(KCH):
                            t = osb.tile([128, TPC], BF16, name=f"o{kc}",
                                         tag=f"o{kc}")
                            r0 = (kc % 2) * 128
                            nc.sync.dma_start(
                                out=t, in_=o_out[s][kc // 2, r0:r0 + 128, :])
                            o_sb.append(t)
                        # Wo: h = Wo^T o + bo, feed straight into FFN input
                        for mc in range(KCH):
                            wo_sb = wo_w.tile([128, KCH, 128], BF16,
                                              name="wo", tag="wo", bufs=3)
                            nc.sync.dma_start(out=wo_sb, in_=wo[s][mc])
                            ps = wo_ps.tile([128, TPC], F32, name="h", tag="h")
                            for kc in range(KCH):
                                nc.tensor.matmul(ps, wo_sb[:, kc, :], o_sb[kc],
                                                 start=(kc == 0),
                                                 stop=(kc == KCH - 1))
                            if fp8:
                                nc.vector.tensor_scalar(
                                    h8[mc // 2][:, mc % 2, :], ps,
                                    bo_sb[s][:, mc:mc + 1], FSC,
                                    op0=AL.add, op1=AL.mult)
                            else:
                                ht = ffn_h.tile([128, TPC], BF16, name=f"h{mc}",
                                                tag=f"h{mc}")
                                nc.vector.tensor_scalar_add(
                                    ht, ps, bo_sb[s][:, mc:mc + 1])
                                hT.append(ht)
                    with tc.tile_pool(name="ffn_g", bufs=1) as ffn_g, \
                         tc.tile_pool(name="ffn_w", bufs=3) as ffn_w, \
                         tc.tile_pool(name="ffn_t", bufs=2) as ffn_t, \
                         tc.tile_pool(name="ffn_ps", bufs=2, space="PSUM") as ffn_ps, \
                         tc.tile_pool(name="ffn_ps1", bufs=1, space="PSUM") as ffn_ps1:
                        g_sb = []
                        DR = mybir.MatmulPerfMode.DoubleRow
                        for jc in range(JCH):
                            wdt2 = FP8 if fp8 else BF16
                            wsh = [128, KCH // 2, 2, 128] if fp8 else [128, KCH, 128]
                            w1t = ffn_w.tile(wsh, wdt2, name="w1", tag="w1")
                            nc.sync.dma_start(out=w1t, in_=w1[s][jc])
                            w3t = ffn_w.tile(wsh, wdt2, name="w3", tag="w3")
                            nc.sync.dma_start(out=w3t, in_=w3[s][jc])
                            z1 = ffn_ps.tile([128, TPC], F32, name="z1", tag="z1")
                            z3 = ffn_ps.tile([128, TPC], F32, name="z3", tag="z3")
                            if fp8:
                                for kp in range(KCH // 2):
                                    nc.tensor.matmul(z1, w1t[:, kp, :, :], h8[kp],
                                                     start=(kp == 0),
                                                     stop=(kp == KCH // 2 - 1),
                                                     perf_mode=DR)
                                for kp in range(KCH // 2):
                                    nc.tensor.matmul(z3, w3t[:, kp, :, :], h8[kp],
                                                     start=(kp == 0),
                                                     stop=(kp == KCH // 2 - 1),
                                                     perf_mode=DR)
                            else:
                                for kc in range(KCH):
                                    nc.tensor.matmul(z1, w1t[:, kc, :], hT[kc],
                                                     start=(kc == 0), stop=(kc == KCH - 1))
                                for kc in range(KCH):
                                    nc.tensor.matmul(z3, w3t[:, kc, :], hT[kc],
                                                     start=(kc == 0), stop=(kc == KCH - 1))
                            sz = ffn_t.tile([128, TPC], F32, name="sz", tag="sz")
                            nc.scalar.activation(out=sz, in_=z1, func=AF.Silu,
                                                 bias=b1_sb[s][:, jc:jc + 1],
                                                 scale=1.0 / (FSC * FSC) if fp8 else 1.0)
                            gt = ffn_g.tile([128, TPC], BF16, name=f"g{jc}",
                                            tag="gt" if fp8 else f"g{jc}",
                                            bufs=2 if fp8 else None)
                            nc.vector.scalar_tensor_tensor(
                                out=gt, in0=z3, scalar=b3_sb[s][:, jc:jc + 1], in1=sz,
                                op0=AL.add, op1=AL.mult)
                            if fp8:
                                if jc % 2 == 0:
                                    g8 = ffn_g.tile([128, 2, TPC], FP8,
                                                    name=f"g8_{jc // 2}",
                                                    tag=f"g8_{jc // 2}")
                                    g_sb.append(g8)
                                nc.scalar.mul(out=g_sb[jc // 2][:, jc % 2, :],
                                              in_=gt, mul=1.0 / FSC)
                            else:
                                g_sb.append(gt)
                        # W2 pass + residual + stats
                        ffr = []
                        ms_ps = ffn_ps1.tile([1, TPC], F32, name="ms", tag="ms")
                        for kc in range(KCH):
                            wsh2 = [128, JCH // 2, 2, 128] if fp8 else [128, JCH, 128]
                            w2t = ffn_w.tile(wsh2, FP8 if fp8 else BF16,
                                             name="w2", tag="w2", bufs=2)
                            nc.sync.dma_start(out=w2t, in_=w2[s][kc])
                            ff = ffn_ps.tile([128, TPC], F32, name="ff", tag="ff")
                            if fp8:
                                for jp in range(JCH // 2):
                                    nc.tensor.matmul(ff, w2t[:, jp, :, :], g_sb[jp],
                                                     start=(jp == 0),
                                                     stop=(jp == JCH // 2 - 1),
                                                     perf_mode=DR)
                            else:
                                for jc in range(JCH):
                                    nc.tensor.matmul(ff, w2t[:, jc, :], g_sb[jc],
                                                     start=(jc == 0), stop=(jc == JCH - 1))
                            xr = ffn_t.tile([128, TPC], F32, name="xr", tag="xr")
                            nc.sync.dma_start(out=xr, in_=xT[s][kc * 128:(kc + 1) * 128, :])
                            fr = ffn_h.tile([128, TPC], F32, name=f"fr{kc}", tag=f"fr{kc}")
                            if fp8:
                                xr2 = ffn_t.tile([128, TPC], F32, name="xr2", tag="xr2")
                                nc.vector.tensor_scalar_add(
                                    xr2, xr, b2_sb[s][:, kc:kc + 1])
                                nc.vector.scalar_tensor_tensor(
                                    out=fr, in0=ff, scalar=1.0 / (FSC * FSC), in1=xr2,
                                    op0=AL.mult, op1=AL.add)
                            else:
                                nc.vector.scalar_tensor_tensor(
                                    out=fr, in0=ff, scalar=b2_sb[s][:, kc:kc + 1],
                                    in1=xr, op0=AL.add, op1=AL.add)
                            ffr.append(fr)
                            sq = ffn_t.tile([128, TPC], F32, name="fsq", tag="fsq")
                            nc.scalar.activation(out=sq, in_=fr, func=AF.Square)
                            nc.tensor.matmul(ms_ps, ones_f, sq,
                                             start=(kc == 0), stop=(kc == KCH - 1))
                        sd = ffn_t.tile([1, TPC], F32, name="fsd", tag="fsd")
                        nc.scalar.activation(out=sd, in_=ms_ps, func=AF.Sqrt,
                                             bias=eps_sb[0:1, :], scale=1.0 / D)
                        rec = ffn_t.tile([1, TPC], F32, name="frec", tag="frec")
                        nc.vector.reciprocal(out=rec, in_=sd)
                        rb_ps = ffn_ps1.tile([128, TPC], F32, name="frb", tag="frb")
                        nc.tensor.matmul(rb_ps, ones_row, rec, start=True, stop=True)
                        rb = ffn_t.tile([128, TPC], F32, name="frbs", tag="frbs")
                        nc.scalar.copy(out=rb, in_=rb_ps)
                        si = 0 if s == "x" else 1
                        for kc in range(KCH):
                            ot = ffn_t.tile([128, TPC], F32, name="ot", tag="ot")
                            nc.vector.scalar_tensor_tensor(
                                out=ot, in0=ffr[kc], scalar=fnw_sb[s][:, kc:kc + 1],
                                in1=rb, op0=AL.mult, op1=AL.mult)
                            nc.sync.dma_start(
                                out=out_ext[si, kc * 128:(kc + 1) * 128, :], in_=ot)
                    side_es.close()

    nc.compile()
    return nc


def prepare_in_maps(inputs):
    perm = _rope_perm()
    x = np.asarray(inputs["x"], np.float32).reshape(T, D)
    y = np.asarray(inputs["y"], np.float32).reshape(T, D)
    cos = np.asarray(inputs["freqs_cos"], np.float32).T  # [64, S]
    sin = np.asarray(inputs["freqs_sin"], np.float32).T
    cs = np.concatenate([cos, cos], 0)                   # [128, S]
    sn = np.concatenate([-sin, sin], 0)
    sc = 1.0 / math.sqrt(HD)

    common = {
        "anw": np.asarray(inputs["attn_norm_w"], np.float32).reshape(KCH, 128),
    }

    def tile_lhs(w):  # [K, M] -> [M//128, 128(part=K%), K//128, 128] tiles
        K, M = w.shape
        return np.ascontiguousarray(
            w.reshape(K // 128, 128, M // 128, 128).transpose(2, 1, 0, 3)
        ).astype(BF)

    for s in ("x", "y"):
        Wq = np.asarray(inputs[f"Wq_{s}"], np.float32)
        Wk = np.asarray(inputs[f"Wk_{s}"], np.float32)
        Wv = np.asarray(inputs[f"Wv_{s}"], np.float32)
        Wo = np.asarray(inputs[f"Wo_{s}"], np.float32)
        bqv = np.asarray(inputs[f"bq_{s}"], np.float32)
        bkv = np.asarray(inputs[f"bk_{s}"], np.float32)

        def tile_col(w):  # [2048, 128] -> [128(part=K%), KCH, 128]
            return np.ascontiguousarray(
                w.reshape(KCH, 128, 128).transpose(1, 0, 2)).astype(BF)

        common[f"wq_{s}"] = np.stack(
            [tile_col(Wq[:, h * HD:(h + 1) * HD][:, perm]) for h in range(H)])
        common[f"wk_{s}"] = np.stack(
            [tile_col(Wk[:, h * HD:(h + 1) * HD][:, perm]) for h in range(H)])
        common[f"wv_{s}"] = np.stack(
            [np.ascontiguousarray(
                Wv[:, dp * 512:(dp + 1) * 512]
                .reshape(KCH, 128, 512).transpose(1, 0, 2)).astype(BF)
             for dp in range(4)])
        common[f"wo_{s}"] = tile_lhs(Wo)
        bq_p = np.stack([bqv[h * HD:(h + 1) * HD][perm] for h in range(H)])
        bk_p = np.stack([bkv[h * HD:(h + 1) * HD][perm] for h in range(H)])
        common[f"bq_{s}"] = bq_p
        common[f"bqs_{s}"] = np.concatenate([bq_p[:, 64:], bq_p[:, :64]], 1)
        common[f"bk_{s}"] = bk_p
        common[f"bks_{s}"] = np.concatenate([bk_p[:, 64:], bk_p[:, :64]], 1)
        common[f"bv_{s}"] = np.asarray(inputs[f"bv_{s}"], np.float32)

        if USE_FP8:
            def tile_f8(w):
                K_, M_ = w.shape
                return np.ascontiguousarray(
                    (w * FSC).reshape(K_ // 128, 128, M_ // 128, 128)
                    .transpose(2, 1, 0, 3)).astype(E4)
            common[f"w1_{s}"] = tile_f8(np.asarray(inputs[f"W1_{s}"], np.float32))
            common[f"w3_{s}"] = tile_f8(np.asarray(inputs[f"W3_{s}"], np.float32))
            common[f"w2_{s}"] = tile_f8(np.asarray(inputs[f"W2_{s}"], np.float32))
        else:
            common[f"w1_{s}"] = tile_lhs(np.asarray(inputs[f"W1_{s}"], np.float32))
            common[f"w3_{s}"] = tile_lhs(np.asarray(inputs[f"W3_{s}"], np.float32))
            common[f"w2_{s}"] = tile_lhs(np.asarray(inputs[f"W2_{s}"], np.float32))
        common[f"b1_{s}"] = np.asarray(inputs[f"b1_{s}"], np.float32).reshape(JCH, 128)
        common[f"b3_{s}"] = np.asarray(inputs[f"b3_{s}"], np.float32).reshape(JCH, 128)
        if USE_FP8:
            common[f"b3_{s}"] = common[f"b3_{s}"] * (FSC * FSC)
        common[f"b2_{s}"] = np.asarray(inputs[f"b2_{s}"], np.float32).reshape(KCH, 128)
        common[f"bo_{s}"] = np.asarray(inputs[f"bo_{s}"], np.float32).reshape(KCH, 128)
        common[f"fnw_{s}"] = np.asarray(
            inputs[f"ffn_norm_w_{s}"], np.float32).reshape(KCH, 128)

    in_maps = []
    for c in range(NC):
        m = dict(common)
        m["xT"] = np.ascontiguousarray(x[c * TPC:(c + 1) * TPC].T)
        m["yT"] = np.ascontiguousarray(y[c * TPC:(c + 1) * TPC].T)
        pos = (c * TPC) % S
        m["cs_q"] = np.ascontiguousarray(cs[:, pos:pos + TPC] * sc)
        m["sn_q"] = np.ascontiguousarray(sn[:, pos:pos + TPC] * sc)
        m["cs_k"] = np.ascontiguousarray(cs[:, pos:pos + TPC])
        m["sn_k"] = np.ascontiguousarray(sn[:, pos:pos + TPC])
        in_maps.append(m)
    return in_maps


def get_nc():
    if "nc" not in _CACHE:
        _CACHE["nc"] = build_nc()
    return _CACHE["nc"]


def kernel(**inputs):
    nc = get_nc()
    in_maps = prepare_in_maps(inputs)
    res = run_bass_kernel_spmd(nc, in_maps, core_ids=list(range(NC)))
    outs = []
    for si in range(2):
        full = np.concatenate([r["out"][si] for r in res.results], axis=1)  # [D, T]
        outs.append(np.ascontiguousarray(full.T).reshape(B, S, D))
    return outs[0], outs[1]


if __name__ == "__main__":
    nc = get_nc()
    print("build + compile OK")
